# revision 31
# baseline (speedup 1.0000x reference)
"""Trainium2 Bass kernel for GQA attention (B=2, T=2048, C=4096, H=32, KV=8, D=128)
with RoPE and causal mask.

Sharding: tensor-parallel over heads across 8 cores. Each core owns 4 Q heads and
their shared KV head: projects q/k/v for those heads, runs causal attention, and
computes a partial output projection; the host sums the 8 partials (bf16 partials,
f32 accumulation on host).

All on-chip layouts are transposed ([feature, token]) so every matmul consumes
natural slices:
  qT/kT/vT = W^T @ x  via lhsT=W-tile [128c, cols], rhs=xT-tile [128c, 512t]
  sT[tk, tq] = kT-tile^T @ qT-chunk   (per 128-row key tile x 512-col query chunk;
               diagonal tiles stream only their unmasked column range)
  pT = exp(sT/sqrt(D) - 10) on ACT; strictly-causal-upper tiles skipped entirely
  S  = sum_k pT  accumulated on DVE (bf16) -> one ones-matmul per (b,h,j) gives
       the softmax denominator broadcast in PSUM (replaces a ones-matmul per tile)
  yT[d, tq] += v-tile^T @ pT          (v laid out [t, d] via DMA-crossbar transpose)
  out[tq, :] += yT_h^T @ wo_h         (accumulate 4 heads in PSUM, evict bf16, DMA)

Phase order is P(b0) P(b1) A(b0) A(b1) with double-buffered qT/kT/vsb so the PE
never sees a projection<->attention boundary stall. Output-projection matmul
"jobs" are popped from a queue inside the attention streams to keep the in-order
PE queue dense while ACT works through the exps.
"""

import os
from collections import deque
from contextlib import ExitStack

import numpy as np
import ml_dtypes

import concourse.bacc as bacc
import concourse.mybir as mybir
import concourse.tile as tile

BF = mybir.dt.bfloat16
F32 = mybir.dt.float32
AFT = mybir.ActivationFunctionType

NCORES = 8
B, T, C = 2, 2048, 4096
H, KV, D = 32, 8, 128
QH = H // NCORES          # 4 q-heads per core
CT = C // 128             # 32 contraction tiles
NCH = T // 512            # 4 query chunks per batch
SCALE = 1.0 / float(np.sqrt(D))
EXP_BIAS = -10.0
ROPE_BASE = 10000.0

bf16 = ml_dtypes.bfloat16


def emit_program():
    nc = bacc.Bacc("TRN2", target_bir_lowering=False, debug=False,
                   num_devices=NCORES)

    # x tiled [c-tile, token-chunk, 128, 512] so every xt DMA is one
    # contiguous 128KB block; out tiled [row-tile, col-pair, 128, 1024] so
    # every store is one contiguous 256KB block (host reassembles)
    xT_d = nc.dram_tensor("xTt", [CT, B * T // 512, 128, 512], BF,
                          kind="ExternalInput").ap()
    wq_d = nc.dram_tensor("wq", [128, CT, QH * D], BF, kind="ExternalInput").ap()
    wk_d = nc.dram_tensor("wk", [128, CT, D], BF, kind="ExternalInput").ap()
    wv_d = nc.dram_tensor("wv", [128, CT, D], BF, kind="ExternalInput").ap()
    wo_d = nc.dram_tensor("woA", [128, QH, C], BF, kind="ExternalInput").ap()
    cos_d = nc.dram_tensor("cosT", [D, T], BF, kind="ExternalInput").ap()
    sin_d = nc.dram_tensor("sinTr", [D, T], BF, kind="ExternalInput").ap()
    alw_d = nc.dram_tensor("allowA", [128, 4, 512], BF, kind="ExternalInput").ap()
    out_d = nc.dram_tensor("out", [B * T // 128, C // 1024, 128, 1024], BF,
                           kind="ExternalOutput").ap()

    with tile.TileContext(nc) as tc, ExitStack() as ctx:
        const = ctx.enter_context(tc.tile_pool(name="const", bufs=1))
        act = ctx.enter_context(tc.tile_pool(name="act", bufs=1))
        work = ctx.enter_context(tc.tile_pool(name="work", bufs=1))

        # weights + tables on the gpsimd DMA queue so they never sit ahead of
        # the xt activation loads (sync queue); chunked in 8-c-tile groups so
        # the first projection matmuls wait on ~1.5MB, not the full tensors.
        # cos/sin follow the first group (needed at the first rope evict); the
        # big wo tensor is emitted after P(b0) so it trickles in last.
        wq_sb = const.tile([128, CT, QH * D], BF)
        wk_sb = const.tile([128, CT, D], BF)
        wv_sb = const.tile([128, CT, D], BF)
        cos_sb = const.tile([D, T], BF)
        sin_sb = const.tile([D, T], BF)
        # finer groups early so the first matmuls start within ~2us, and the
        # rope tables only load once half the weights are in
        for g0, g1 in ((0, 1), (1, 2), (2, 4), (4, 8), (8, 16), (16, 24), (24, 32)):
            s = slice(g0, g1)
            nc.gpsimd.dma_start(wq_sb[:, s, :], wq_d[:, s, :])
            nc.gpsimd.dma_start(wk_sb[:, s, :], wk_d[:, s, :])
            nc.gpsimd.dma_start(wv_sb[:, s, :], wv_d[:, s, :])
            if g1 == 24:
                nc.gpsimd.dma_start(cos_sb[:], cos_d)
                nc.gpsimd.dma_start(sin_sb[:], sin_d)
        alw_sb = const.tile([128, 4, 512], BF)
        wo_sb = const.tile([128, QH, C], BF)
        onesbf_sb = const.tile([128, 128], BF)
        nc.gpsimd.memset(onesbf_sb[:], 1.0)
        bias_sb = const.tile([128, 1], F32)
        nc.gpsimd.memset(bias_sb[:], EXP_BIAS)

        def rope_sb(dst, src, cs):
            # dst = src * cos + swap_halves(src) * sin_rot   (all bf16 SBUF so
            # DVE runs in 2x/4x perf modes; src was evicted from PSUM by ACT)
            sw = work.tile([128, 512], BF, tag="sw", bufs=2, name="sw")
            nc.vector.tensor_copy(sw[0:64, :], src[64:128, :])
            nc.vector.tensor_copy(sw[64:128, :], src[0:64, :])
            nc.vector.tensor_mul(sw[:], sw[:], sin_sb[:, cs])
            cst = work.tile([128, 512], BF, tag="cst", bufs=2, name="cst")
            nc.vector.tensor_mul(cst[:], src[:], cos_sb[:, cs])
            nc.vector.tensor_add(dst, cst[:], sw[:])

        tr_pending = deque()
        rope_pending = deque()

        def flush_tr(n):
            for _ in range(min(n, len(tr_pending))):
                tr_pending.popleft()()

        def flush_rope(n):
            for _ in range(min(n, len(rope_pending))):
                rope_pending.popleft()()

        def proj_batch(pp, b):
            qT = act.tile([D, QH, T], BF, tag="qT", bufs=2, name="qT")
            kT = act.tile([D, T], BF, tag="kT", bufs=2, name="kT")
            vT = act.tile([D, T], BF, tag="vT", bufs=1, name="vT")
            vsb = act.tile([128, T // 128, D], BF, tag="v", bufs=2, name="vsb")
            for jc in range(NCH):
                pq = [pp.tile([128, 512], F32, tag=f"pq{h}", name=f"pq{h}")
                      for h in range(QH)]
                pk = pp.tile([128, 512], F32, tag="pk", bufs=2, name="pk")
                pv = pp.tile([128, 512], F32, tag="pv", bufs=2, name="pv")
                # q matmuls run SKEW c-tiles behind k/v so the previous
                # chunk's pq bank evictions are hidden behind ready work;
                # deeper skew in the very first chunk relaxes the deadline on
                # the tail weight groups while HBM is still loading them
                SKEW = 12 if (b == 0 and jc == 0) else 4
                xts = {}
                cc = (b * T + 512 * jc) // 512

                def q_mms(cq, h):
                    nc.tensor.matmul(
                        pq[h][:], wq_sb[:, cq, 128 * h:128 * (h + 1)],
                        xts[cq][:], start=cq == 0, stop=cq == CT - 1)

                for ci in range(CT):
                    # flush the previous chunk's deferred v-transpose here so
                    # it sits behind this chunk's first xt loads on the sync
                    # queue (emitted at the top it would head-of-line block
                    # the prefetch while waiting for the vT eviction)
                    if ci == 8:
                        flush_tr(4)
                    xt = work.tile([128, 512], BF, tag="xt", bufs=17, name="xt")
                    xts[ci] = xt
                    nc.sync.dma_start(xt[:], xT_d[ci, cc, :, :])
                    st, sp = ci == 0, ci == CT - 1
                    nc.tensor.matmul(pk[:], wk_sb[:, ci, :], xt[:],
                                     start=st, stop=sp)
                    nc.tensor.matmul(pv[:], wv_sb[:, ci, :], xt[:],
                                     start=st, stop=sp)
                    if ci >= SKEW:
                        for h in range(QH):
                            q_mms(ci - SKEW, h)
                        del xts[ci - SKEW]
                cs = slice(512 * jc, 512 * (jc + 1))
                # fast ACT copies free the PSUM banks so the next chunk's
                # matmuls never wait on the rope math (which runs SBUF-side
                # on DVE afterwards); kraw/vT copies overlap the q tail, and
                # the head-major tail staggers the pq stops so the qraw
                # copies pipeline against remaining tail matmuls
                kraw = work.tile([128, 512], BF, tag="kraw", bufs=2,
                                 name="kraw")
                nc.scalar.copy(kraw[:], pk[:])
                nc.scalar.copy(vT[:, cs], pv[:])
                qraws = []
                for h in range(QH):
                    for cq in range(CT - SKEW, CT):
                        q_mms(cq, h)
                    qraw = work.tile([128, 512], BF, tag="qraw", bufs=5,
                                     name="qraw")
                    nc.scalar.copy(qraw[:], pq[h][:])
                    qraws.append(qraw)
                # the very last chunk's ropes are deferred into the
                # attention phase: they are only read by A(b1), and emitted
                # here they would sit ahead of A(b0)'s mask-muls in the DVE
                # queue, stalling the first attention groups
                if b == 1 and jc == NCH - 1:
                    rope_pending.append(
                        lambda kT=kT, kraw=kraw, cs=cs: rope_sb(kT[:, cs], kraw, cs))
                    for h in range(QH):
                        rope_pending.append(
                            lambda qT=qT, h=h, q=qraws[h], cs=cs:
                                rope_sb(qT[:, h, cs], q, cs))
                else:
                    rope_sb(kT[:, cs], kraw, cs)
                    for h in range(QH):
                        rope_sb(qT[:, h, cs], qraws[h], cs)
                # v chunk -> [t, d] tiles via the DMA crossbar: one transpose
                # per chunk ([128,512] -> [128,4,128] maps vsb[p,k,d] =
                # vT[d,128k+p]), on the sync hwdge queue so its descriptor
                # cost never delays the ACT bank-freeing copies; deferred one
                # chunk so it cannot sit ahead of this chunk's copies
                tr_pending.append(
                    lambda jc=jc, cs=cs, vsb=vsb, vT=vT:
                        nc.sync.dma_start_transpose(
                            vsb[:, 4 * jc:4 * jc + 4, :], vT[:, cs]))
            return qT, kT, vsb

        with tc.tile_pool(name="pproj", bufs=1, space="PSUM") as pp:
            acts0 = proj_batch(pp, 0)
            # gate the wo/alw loads on a DVE op emitted here so the DMA
            # cannot start before P(b0) finishes -- keeps the HBM free for
            # the xt stream during the warmup chunks (both are only read in
            # the attention phase)
            nc.vector.memset(wo_sb[:, 0, 0:8], 0.0)
            nc.vector.memset(alw_sb[:, 0, 0:8], 0.0)
            nc.gpsimd.dma_start(alw_sb[:], alw_d)
            nc.gpsimd.dma_start(wo_sb[:], wo_d)
            acts1 = proj_batch(pp, 1)

        # ---- attention + output projection ----
        with tc.tile_pool(name="pattn", bufs=1, space="PSUM") as pa:
            wo_jobs = deque()
            drain_mode = [False]

            def make_wo_job(b, j, tl, op, yts):
                # one job covers two adjacent 512-col output slices so the
                # store DMA gets 2KB lines (one [128,1024] bf16 transfer)
                def job():
                    ob = work.tile([128, 1024], BF, tag="ob", bufs=3,
                                   name="ob")
                    for half in range(2):
                        o = 2 * op + half
                        ops = pa.tile([128, 512], F32, tag="ops", bufs=2,
                                      name="ops")
                        for h in range(QH):
                            nc.tensor.matmul(
                                ops[:], yts[h][:, 128 * tl:128 * (tl + 1)],
                                wo_sb[:, h, 512 * o:512 * (o + 1)],
                                start=h == 0, stop=h == QH - 1)
                        # spread the PSUM eviction across ACT and DVE:
                        # alternate except in the j==3 in-stream portion
                        # where ACT is saturated by exps (in the end-drain
                        # ACT is idle, so alternate there too)
                        if half == 0 and (j < 3 or drain_mode[0]):
                            nc.scalar.copy(ob[:, 0:512], ops[:])
                        else:
                            nc.vector.tensor_copy(ob[:, 512 * half:512 * (half + 1)], ops[:])
                    rt = (b * T + 512 * j + 128 * tl) // 128
                    nc.sync.dma_start(out_d[rt, op, :, :], ob[:])
                return job

            for b, (qT, kT, vsb) in ((0, acts0), (1, acts1)):
                for j in range(NCH):
                    yts = {}
                    for h in range(QH):
                        # sprinkle the last projection chunk's v transposes
                        # and deferred ropes into the early attention heads
                        flush_tr(1)
                        if not (b == 0 and j == 0 and h == 0):
                            flush_rope(1)
                        yps = pa.tile([128, 512], F32, tag="yps", bufs=1,
                                      name="yps")
                        K = 4 * j + 4
                        # pass 1: score matmuls stream; exp/mask/denominator
                        # trail on ACT/DVE. Diagonal tiles (o>=1) only touch
                        # their unmasked column range [128*o:512].
                        S = work.tile([128, 512], BF, tag="S", bufs=2,
                                      name="S")
                        pts = []
                        for k in range(K):
                            o = k - 4 * j
                            c0 = 128 * o if o > 0 else 0
                            sl = slice(c0, 512)
                            sps = pa.tile([128, 512], F32, tag="sps", bufs=4,
                                          name="sps")
                            nc.tensor.matmul(
                                sps[:, sl], kT[:, 128 * k:128 * (k + 1)],
                                qT[:, h, 512 * j + c0:512 * (j + 1)],
                                start=True, stop=True)
                            pt = work.tile([128, 512], BF, tag="pt", bufs=16,
                                           name="pt")
                            nc.scalar.activation(pt[:, sl], sps[:, sl], AFT.Exp,
                                                 bias=bias_sb[:], scale=SCALE)
                            if o >= 0:
                                nc.vector.tensor_mul(pt[:, sl], pt[:, sl],
                                                     alw_sb[:, o, sl])
                            if k == 0:
                                nc.vector.tensor_copy(S[:], pt[:])
                            else:
                                nc.vector.tensor_add(S[:, sl], S[:, sl],
                                                     pt[:, sl])
                            pts.append((pt, sl))
                            if wo_jobs:
                                wo_jobs.popleft()()
                        # pass 2: attn@v accumulation; k=0 always covers the
                        # full 512 columns so the start-matmul initializes the
                        # whole bank
                        for k, (pt, sl) in enumerate(pts):
                            nc.tensor.matmul(yps[:, sl], vsb[:, k, :],
                                             pt[:, sl],
                                             start=(k == 0), stop=(k == K - 1))
                            if wo_jobs:
                                wo_jobs.popleft()()
                        dns = pa.tile([128, 512], F32, tag="dns", bufs=1,
                                      name="dns")
                        nc.tensor.matmul(dns[:], onesbf_sb[:], S[:],
                                         start=True, stop=True)
                        rec = work.tile([128, 512], F32, tag="rec", bufs=1,
                                        name="rec")
                        nc.vector.reciprocal_approx_fast(rec[:], dns[:])
                        yt = work.tile([128, 512], BF, tag="yt", bufs=8,
                                       name="yt")
                        nc.vector.tensor_mul(yt[:], yps[:], rec[:])
                        yts[h] = yt
                    for tl in range(4):
                        for op in range(C // 1024):
                            wo_jobs.append(make_wo_job(b, j, tl, op, yts))
            drain_mode[0] = True
            while wo_jobs:
                wo_jobs.popleft()()

    nc.compile()
    return nc


def host_prep(inputs):
    x = np.asarray(inputs["x"], np.float32)
    mask = np.asarray(inputs["mask"], np.float32)
    wq = np.asarray(inputs["wq"], np.float32)
    wk = np.asarray(inputs["wk"], np.float32)
    wv = np.asarray(inputs["wv"], np.float32)
    wo = np.asarray(inputs["wo"], np.float32)

    xT = x.reshape(B * T, C).T  # [C, B*T]
    xTt = np.ascontiguousarray(
        xT.reshape(CT, 128, B * T // 512, 512).transpose(0, 2, 1, 3)
    ).astype(bf16)  # [ci, chunk, 128, 512] contiguous tiles
    inv = 1.0 / (ROPE_BASE ** (np.arange(0, D, 2, dtype=np.float64) / D))
    freqs = np.arange(T, dtype=np.float64)[:, None] * inv[None, :] * B
    emb = np.concatenate([freqs, freqs], axis=-1)       # [T, D]
    cosT = np.cos(emb).T.astype(np.float32).astype(bf16)
    sinT = np.sin(emb).T.astype(np.float32)
    sinT[: D // 2] *= -1.0
    sinTr = sinT.astype(bf16)
    # allow[p, o, jj] = 1 - mask[jj, 128*o + p]  (from the actual mask input)
    allowA = np.ascontiguousarray(
        np.stack([(1.0 - mask[0:512, 128 * o:128 * (o + 1)]).T
                  for o in range(4)], axis=1)).astype(bf16)   # [128, 4, 512]

    common = dict(xTt=xTt, cosT=cosT, sinTr=sinTr, allowA=allowA)
    in_maps = []
    for c in range(NCORES):
        m = dict(common)
        m["wq"] = np.ascontiguousarray(
            wq[:, 512 * c:512 * (c + 1)].reshape(CT, 128, QH * D)
            .transpose(1, 0, 2)).astype(bf16)
        m["wk"] = np.ascontiguousarray(
            wk[:, 128 * c:128 * (c + 1)].reshape(CT, 128, D)
            .transpose(1, 0, 2)).astype(bf16)
        m["wv"] = np.ascontiguousarray(
            wv[:, 128 * c:128 * (c + 1)].reshape(CT, 128, D)
            .transpose(1, 0, 2)).astype(bf16)
        m["woA"] = np.ascontiguousarray(
            wo[512 * c:512 * (c + 1), :].reshape(QH, 128, C)
            .transpose(1, 0, 2)).astype(bf16)
        in_maps.append(m)
    return in_maps


def kernel(**inputs) -> np.ndarray:
    from concourse.bass_utils import run_bass_kernel_spmd

    in_maps = host_prep(inputs)
    nc = emit_program()
    trace = bool(os.environ.get("BASS_KERNEL_TRACE"))
    res = run_bass_kernel_spmd(nc, in_maps, core_ids=list(range(NCORES)),
                               trace=trace)
    if trace and res.exec_time_ns is not None:
        print(f"HW exec time: {res.exec_time_ns} ns")
        if res.instructions_and_trace is not None:
            print("trace:", res.instructions_and_trace[1])
    total = np.zeros((B * T // 128, C // 1024, 128, 1024), np.float32)
    for r in res.results:
        total += np.asarray(r["out"], dtype=np.float32)
    # untile [row-tile, col-pair, 128, 1024] -> [B*T, C]
    full = total.transpose(0, 2, 1, 3).reshape(B * T, C)
    return np.ascontiguousarray(full).reshape(B, T, C)


# revision 32
# speedup vs baseline: 1.0107x; 1.0107x over previous
"""Trainium2 Bass kernel for GQA attention (B=2, T=2048, C=4096, H=32, KV=8, D=128)
with RoPE and causal mask.

Sharding: tensor-parallel over heads across 8 cores. Each core owns 4 Q heads and
their shared KV head: projects q/k/v for those heads, runs causal attention, and
computes a partial output projection; the host sums the 8 partials (bf16 partials,
f32 accumulation on host).

All on-chip layouts are transposed ([feature, token]) so every matmul consumes
natural slices:
  qT/kT/vT = W^T @ x  via lhsT=W-tile [128c, cols], rhs=xT-tile [128c, 512t]
  sT[tk, tq] = kT-tile^T @ qT-chunk   (per 128-row key tile x 512-col query chunk;
               diagonal tiles stream only their unmasked column range)
  pT = exp(sT/sqrt(D) - 10) on ACT; strictly-causal-upper tiles skipped entirely
  S  = sum_k pT  accumulated on DVE (bf16) -> one ones-matmul per (b,h,j) gives
       the softmax denominator broadcast in PSUM (replaces a ones-matmul per tile)
  yT[d, tq] += v-tile^T @ pT          (v laid out [t, d] via DMA-crossbar transpose)
  out[tq, :] += yT_h^T @ wo_h         (accumulate 4 heads in PSUM, evict bf16, DMA)

Phase order is P(b0) P(b1) A(b0) A(b1) with double-buffered qT/kT/vsb so the PE
never sees a projection<->attention boundary stall. Output-projection matmul
"jobs" are popped from a queue inside the attention streams to keep the in-order
PE queue dense while ACT works through the exps.
"""

import os
from collections import deque
from contextlib import ExitStack

import numpy as np
import ml_dtypes

import concourse.bacc as bacc
import concourse.mybir as mybir
import concourse.tile as tile

BF = mybir.dt.bfloat16
F32 = mybir.dt.float32
AFT = mybir.ActivationFunctionType

NCORES = 8
B, T, C = 2, 2048, 4096
H, KV, D = 32, 8, 128
QH = H // NCORES          # 4 q-heads per core
CT = C // 128             # 32 contraction tiles
NCH = T // 512            # 4 query chunks per batch
SCALE = 1.0 / float(np.sqrt(D))
EXP_BIAS = -10.0
ROPE_BASE = 10000.0

bf16 = ml_dtypes.bfloat16


def emit_program():
    nc = bacc.Bacc("TRN2", target_bir_lowering=False, debug=False,
                   num_devices=NCORES)

    # x tiled [c-tile, token-chunk, 128, 512] so every xt DMA is one
    # contiguous 128KB block; out tiled [row-tile, col-pair, 128, 1024] so
    # every store is one contiguous 256KB block (host reassembles)
    xT_d = nc.dram_tensor("xTt", [CT, B * T // 512, 128, 512], BF,
                          kind="ExternalInput").ap()
    wq_d = nc.dram_tensor("wq", [128, CT, QH * D], BF, kind="ExternalInput").ap()
    wk_d = nc.dram_tensor("wk", [128, CT, D], BF, kind="ExternalInput").ap()
    wv_d = nc.dram_tensor("wv", [128, CT, D], BF, kind="ExternalInput").ap()
    wo_d = nc.dram_tensor("woA", [128, QH, C], BF, kind="ExternalInput").ap()
    cos_d = nc.dram_tensor("cosT", [D, T], BF, kind="ExternalInput").ap()
    sin_d = nc.dram_tensor("sinTr", [D, T], BF, kind="ExternalInput").ap()
    alw_d = nc.dram_tensor("allowA", [128, 4, 512], BF, kind="ExternalInput").ap()
    out_d = nc.dram_tensor("out", [B * T // 128, C // 1024, 128, 1024], BF,
                           kind="ExternalOutput").ap()

    with tile.TileContext(nc) as tc, ExitStack() as ctx:
        const = ctx.enter_context(tc.tile_pool(name="const", bufs=1))
        act = ctx.enter_context(tc.tile_pool(name="act", bufs=1))
        work = ctx.enter_context(tc.tile_pool(name="work", bufs=1))

        # weights + tables on the gpsimd DMA queue so they never sit ahead of
        # the xt activation loads (sync queue); chunked in 8-c-tile groups so
        # the first projection matmuls wait on ~1.5MB, not the full tensors.
        # cos/sin follow the first group (needed at the first rope evict); the
        # big wo tensor is emitted after P(b0) so it trickles in last.
        wq_sb = const.tile([128, CT, QH * D], BF)
        wk_sb = const.tile([128, CT, D], BF)
        wv_sb = const.tile([128, CT, D], BF)
        cos_sb = const.tile([D, T], BF)
        sin_sb = const.tile([D, T], BF)
        # finer groups early so the first matmuls start within ~2us, and the
        # rope tables only load once half the weights are in
        for g0, g1 in ((0, 1), (1, 2), (2, 4), (4, 8), (8, 16), (16, 24), (24, 32)):
            s = slice(g0, g1)
            nc.gpsimd.dma_start(wq_sb[:, s, :], wq_d[:, s, :])
            nc.gpsimd.dma_start(wk_sb[:, s, :], wk_d[:, s, :])
            nc.gpsimd.dma_start(wv_sb[:, s, :], wv_d[:, s, :])
            if g1 == 24:
                nc.gpsimd.dma_start(cos_sb[:], cos_d)
                nc.gpsimd.dma_start(sin_sb[:], sin_d)
        alw_sb = const.tile([128, 4, 512], BF)
        wo_sb = const.tile([128, QH, C], BF)
        onesbf_sb = const.tile([128, 128], BF)
        nc.gpsimd.memset(onesbf_sb[:], 1.0)
        bias_sb = const.tile([128, 1], F32)
        nc.gpsimd.memset(bias_sb[:], EXP_BIAS)

        def rope_sb(dst, src, cs):
            # dst = src * cos + swap_halves(src) * sin_rot   (all bf16 SBUF so
            # DVE runs in 2x/4x perf modes; src was evicted from PSUM by ACT)
            sw = work.tile([128, 512], BF, tag="sw", bufs=2, name="sw")
            nc.vector.tensor_copy(sw[0:64, :], src[64:128, :])
            nc.vector.tensor_copy(sw[64:128, :], src[0:64, :])
            nc.vector.tensor_mul(sw[:], sw[:], sin_sb[:, cs])
            cst = work.tile([128, 512], BF, tag="cst", bufs=2, name="cst")
            nc.vector.tensor_mul(cst[:], src[:], cos_sb[:, cs])
            nc.vector.tensor_add(dst, cst[:], sw[:])

        tr_pending = deque()
        rope_pending = deque()

        def flush_tr(n):
            for _ in range(min(n, len(tr_pending))):
                tr_pending.popleft()()

        def flush_rope(n):
            for _ in range(min(n, len(rope_pending))):
                rope_pending.popleft()()

        def proj_batch(pp, b):
            qT = act.tile([D, QH, T], BF, tag="qT", bufs=2, name="qT")
            kT = act.tile([D, T], BF, tag="kT", bufs=2, name="kT")
            vT = act.tile([D, T], BF, tag="vT", bufs=1, name="vT")
            vsb = act.tile([128, T // 128, D], BF, tag="v", bufs=2, name="vsb")
            for jc in range(NCH):
                pq = [pp.tile([128, 512], F32, tag=f"pq{h}", name=f"pq{h}")
                      for h in range(QH)]
                pk = pp.tile([128, 512], F32, tag="pk", bufs=2, name="pk")
                pv = pp.tile([128, 512], F32, tag="pv", bufs=2, name="pv")
                # q matmuls run SKEW c-tiles behind k/v so the previous
                # chunk's pq bank evictions are hidden behind ready work;
                # deeper skew in the very first chunk relaxes the deadline on
                # the tail weight groups while HBM is still loading them
                SKEW = 4
                xts = {}
                cc = (b * T + 512 * jc) // 512

                def q_mms(cq, h):
                    nc.tensor.matmul(
                        pq[h][:], wq_sb[:, cq, 128 * h:128 * (h + 1)],
                        xts[cq][:], start=cq == 0, stop=cq == CT - 1)

                for ci in range(CT):
                    # flush the previous chunk's deferred v-transpose here so
                    # it sits behind this chunk's first xt loads on the sync
                    # queue (emitted at the top it would head-of-line block
                    # the prefetch while waiting for the vT eviction)
                    if ci == 8:
                        flush_tr(4)
                    xt = work.tile([128, 512], BF, tag="xt", bufs=17, name="xt")
                    xts[ci] = xt
                    nc.sync.dma_start(xt[:], xT_d[ci, cc, :, :])
                    st, sp = ci == 0, ci == CT - 1
                    nc.tensor.matmul(pk[:], wk_sb[:, ci, :], xt[:],
                                     start=st, stop=sp)
                    nc.tensor.matmul(pv[:], wv_sb[:, ci, :], xt[:],
                                     start=st, stop=sp)
                    if ci >= SKEW:
                        for h in range(QH):
                            q_mms(ci - SKEW, h)
                        del xts[ci - SKEW]
                cs = slice(512 * jc, 512 * (jc + 1))
                # fast ACT copies free the PSUM banks so the next chunk's
                # matmuls never wait on the rope math (which runs SBUF-side
                # on DVE afterwards); kraw/vT copies overlap the q tail, and
                # the head-major tail staggers the pq stops so the qraw
                # copies pipeline against remaining tail matmuls
                kraw = work.tile([128, 512], BF, tag="kraw", bufs=2,
                                 name="kraw")
                nc.scalar.copy(kraw[:], pk[:])
                nc.scalar.copy(vT[:, cs], pv[:])
                qraws = []
                for h in range(QH):
                    for cq in range(CT - SKEW, CT):
                        q_mms(cq, h)
                    qraw = work.tile([128, 512], BF, tag="qraw", bufs=5,
                                     name="qraw")
                    nc.scalar.copy(qraw[:], pq[h][:])
                    qraws.append(qraw)
                # the very last chunk's ropes are deferred into the
                # attention phase: they are only read by A(b1), and emitted
                # here they would sit ahead of A(b0)'s mask-muls in the DVE
                # queue, stalling the first attention groups
                if b == 1 and jc == NCH - 1:
                    rope_pending.append(
                        lambda kT=kT, kraw=kraw, cs=cs: rope_sb(kT[:, cs], kraw, cs))
                    for h in range(QH):
                        rope_pending.append(
                            lambda qT=qT, h=h, q=qraws[h], cs=cs:
                                rope_sb(qT[:, h, cs], q, cs))
                else:
                    rope_sb(kT[:, cs], kraw, cs)
                    for h in range(QH):
                        rope_sb(qT[:, h, cs], qraws[h], cs)
                # v chunk -> [t, d] tiles via the DMA crossbar: one transpose
                # per chunk ([128,512] -> [128,4,128] maps vsb[p,k,d] =
                # vT[d,128k+p]), on the sync hwdge queue so its descriptor
                # cost never delays the ACT bank-freeing copies; deferred one
                # chunk so it cannot sit ahead of this chunk's copies
                tr_pending.append(
                    lambda jc=jc, cs=cs, vsb=vsb, vT=vT:
                        nc.sync.dma_start_transpose(
                            vsb[:, 4 * jc:4 * jc + 4, :], vT[:, cs]))
            return qT, kT, vsb

        with tc.tile_pool(name="pproj", bufs=1, space="PSUM") as pp:
            acts0 = proj_batch(pp, 0)
            # gate the wo/alw loads on a DVE op emitted here so the DMA
            # cannot start before P(b0) finishes -- keeps the HBM free for
            # the xt stream during the warmup chunks (both are only read in
            # the attention phase)
            nc.vector.memset(wo_sb[:, 0, 0:8], 0.0)
            nc.vector.memset(alw_sb[:, 0, 0:8], 0.0)
            nc.gpsimd.dma_start(alw_sb[:], alw_d)
            nc.gpsimd.dma_start(wo_sb[:], wo_d)
            acts1 = proj_batch(pp, 1)

        # ---- attention + output projection ----
        with tc.tile_pool(name="pattn", bufs=1, space="PSUM") as pa:
            wo_jobs = deque()
            drain_mode = [False]

            def make_wo_job(b, j, tl, op, yts):
                # one job covers two adjacent 512-col output slices so the
                # store DMA gets 2KB lines (one [128,1024] bf16 transfer)
                def job():
                    ob = work.tile([128, 1024], BF, tag="ob", bufs=3,
                                   name="ob")
                    for half in range(2):
                        o = 2 * op + half
                        ops = pa.tile([128, 512], F32, tag="ops", bufs=2,
                                      name="ops")
                        for h in range(QH):
                            nc.tensor.matmul(
                                ops[:], yts[h][:, 128 * tl:128 * (tl + 1)],
                                wo_sb[:, h, 512 * o:512 * (o + 1)],
                                start=h == 0, stop=h == QH - 1)
                        # spread the PSUM eviction across ACT and DVE:
                        # alternate except in the j==3 in-stream portion
                        # where ACT is saturated by exps (in the end-drain
                        # ACT is idle, so alternate there too)
                        if half == 0 and (j < 3 or drain_mode[0]):
                            nc.scalar.copy(ob[:, 0:512], ops[:])
                        else:
                            nc.vector.tensor_copy(ob[:, 512 * half:512 * (half + 1)], ops[:])
                    rt = (b * T + 512 * j + 128 * tl) // 128
                    nc.sync.dma_start(out_d[rt, op, :, :], ob[:])
                return job

            for b, (qT, kT, vsb) in ((0, acts0), (1, acts1)):
                for j in range(NCH):
                    yts = {}
                    for h in range(QH):
                        # sprinkle the last projection chunk's v transposes
                        # and deferred ropes into the early attention heads
                        flush_tr(1)
                        if not (b == 0 and j == 0 and h == 0):
                            flush_rope(1)
                        yps = pa.tile([128, 512], F32, tag="yps", bufs=1,
                                      name="yps")
                        K = 4 * j + 4
                        # pass 1: score matmuls stream; exp/mask/denominator
                        # trail on ACT/DVE. Diagonal tiles (o>=1) only touch
                        # their unmasked column range [128*o:512].
                        S = work.tile([128, 512], BF, tag="S", bufs=2,
                                      name="S")
                        pts = []
                        for k in range(K):
                            o = k - 4 * j
                            c0 = 128 * o if o > 0 else 0
                            sl = slice(c0, 512)
                            sps = pa.tile([128, 512], F32, tag="sps", bufs=4,
                                          name="sps")
                            nc.tensor.matmul(
                                sps[:, sl], kT[:, 128 * k:128 * (k + 1)],
                                qT[:, h, 512 * j + c0:512 * (j + 1)],
                                start=True, stop=True)
                            pt = work.tile([128, 512], BF, tag="pt", bufs=16,
                                           name="pt")
                            nc.scalar.activation(pt[:, sl], sps[:, sl], AFT.Exp,
                                                 bias=bias_sb[:], scale=SCALE)
                            if o >= 0:
                                nc.vector.tensor_mul(pt[:, sl], pt[:, sl],
                                                     alw_sb[:, o, sl])
                            if k == 0:
                                nc.vector.tensor_copy(S[:], pt[:])
                            else:
                                nc.vector.tensor_add(S[:, sl], S[:, sl],
                                                     pt[:, sl])
                            pts.append((pt, sl))
                            if wo_jobs:
                                wo_jobs.popleft()()
                        # pass 2: attn@v accumulation; k=0 always covers the
                        # full 512 columns so the start-matmul initializes the
                        # whole bank
                        for k, (pt, sl) in enumerate(pts):
                            nc.tensor.matmul(yps[:, sl], vsb[:, k, :],
                                             pt[:, sl],
                                             start=(k == 0), stop=(k == K - 1))
                            if wo_jobs:
                                wo_jobs.popleft()()
                        dns = pa.tile([128, 512], F32, tag="dns", bufs=1,
                                      name="dns")
                        nc.tensor.matmul(dns[:], onesbf_sb[:], S[:],
                                         start=True, stop=True)
                        rec = work.tile([128, 512], F32, tag="rec", bufs=1,
                                        name="rec")
                        nc.vector.reciprocal_approx_fast(rec[:], dns[:])
                        yt = work.tile([128, 512], BF, tag="yt", bufs=8,
                                       name="yt")
                        nc.vector.tensor_mul(yt[:], yps[:], rec[:])
                        yts[h] = yt
                    for tl in range(4):
                        for op in range(C // 1024):
                            wo_jobs.append(make_wo_job(b, j, tl, op, yts))
            drain_mode[0] = True
            while wo_jobs:
                wo_jobs.popleft()()

    nc.compile()
    return nc


def host_prep(inputs):
    x = np.asarray(inputs["x"], np.float32)
    mask = np.asarray(inputs["mask"], np.float32)
    wq = np.asarray(inputs["wq"], np.float32)
    wk = np.asarray(inputs["wk"], np.float32)
    wv = np.asarray(inputs["wv"], np.float32)
    wo = np.asarray(inputs["wo"], np.float32)

    xT = x.reshape(B * T, C).T  # [C, B*T]
    xTt = np.ascontiguousarray(
        xT.reshape(CT, 128, B * T // 512, 512).transpose(0, 2, 1, 3)
    ).astype(bf16)  # [ci, chunk, 128, 512] contiguous tiles
    inv = 1.0 / (ROPE_BASE ** (np.arange(0, D, 2, dtype=np.float64) / D))
    freqs = np.arange(T, dtype=np.float64)[:, None] * inv[None, :] * B
    emb = np.concatenate([freqs, freqs], axis=-1)       # [T, D]
    cosT = np.cos(emb).T.astype(np.float32).astype(bf16)
    sinT = np.sin(emb).T.astype(np.float32)
    sinT[: D // 2] *= -1.0
    sinTr = sinT.astype(bf16)
    # allow[p, o, jj] = 1 - mask[jj, 128*o + p]  (from the actual mask input)
    allowA = np.ascontiguousarray(
        np.stack([(1.0 - mask[0:512, 128 * o:128 * (o + 1)]).T
                  for o in range(4)], axis=1)).astype(bf16)   # [128, 4, 512]

    common = dict(xTt=xTt, cosT=cosT, sinTr=sinTr, allowA=allowA)
    in_maps = []
    for c in range(NCORES):
        m = dict(common)
        m["wq"] = np.ascontiguousarray(
            wq[:, 512 * c:512 * (c + 1)].reshape(CT, 128, QH * D)
            .transpose(1, 0, 2)).astype(bf16)
        m["wk"] = np.ascontiguousarray(
            wk[:, 128 * c:128 * (c + 1)].reshape(CT, 128, D)
            .transpose(1, 0, 2)).astype(bf16)
        m["wv"] = np.ascontiguousarray(
            wv[:, 128 * c:128 * (c + 1)].reshape(CT, 128, D)
            .transpose(1, 0, 2)).astype(bf16)
        m["woA"] = np.ascontiguousarray(
            wo[512 * c:512 * (c + 1), :].reshape(QH, 128, C)
            .transpose(1, 0, 2)).astype(bf16)
        in_maps.append(m)
    return in_maps


def kernel(**inputs) -> np.ndarray:
    from concourse.bass_utils import run_bass_kernel_spmd

    in_maps = host_prep(inputs)
    nc = emit_program()
    trace = bool(os.environ.get("BASS_KERNEL_TRACE"))
    res = run_bass_kernel_spmd(nc, in_maps, core_ids=list(range(NCORES)),
                               trace=trace)
    if trace and res.exec_time_ns is not None:
        print(f"HW exec time: {res.exec_time_ns} ns")
        if res.instructions_and_trace is not None:
            print("trace:", res.instructions_and_trace[1])
    total = np.zeros((B * T // 128, C // 1024, 128, 1024), np.float32)
    for r in res.results:
        total += np.asarray(r["out"], dtype=np.float32)
    # untile [row-tile, col-pair, 128, 1024] -> [B*T, C]
    full = total.transpose(0, 2, 1, 3).reshape(B * T, C)
    return np.ascontiguousarray(full).reshape(B, T, C)


# revision 33
# speedup vs baseline: 1.0131x; 1.0024x over previous
"""Trainium2 Bass kernel for GQA attention (B=2, T=2048, C=4096, H=32, KV=8, D=128)
with RoPE and causal mask.

Sharding: tensor-parallel over heads across 8 cores. Each core owns 4 Q heads and
their shared KV head: projects q/k/v for those heads, runs causal attention, and
computes a partial output projection; the host sums the 8 partials (bf16 partials,
f32 accumulation on host).

All on-chip layouts are transposed ([feature, token]) so every matmul consumes
natural slices:
  qT/kT/vT = W^T @ x  via lhsT=W-tile [128c, cols], rhs=xT-tile [128c, 512t]
  sT[tk, tq] = kT-tile^T @ qT-chunk   (per 128-row key tile x 512-col query chunk;
               diagonal tiles stream only their unmasked column range)
  pT = exp(sT/sqrt(D) - 10) on ACT; strictly-causal-upper tiles skipped entirely
  S  = sum_k pT  accumulated on DVE (bf16) -> one ones-matmul per (b,h,j) gives
       the softmax denominator broadcast in PSUM (replaces a ones-matmul per tile)
  yT[d, tq] += v-tile^T @ pT          (v laid out [t, d] via DMA-crossbar transpose)
  out[tq, :] += yT_h^T @ wo_h         (accumulate 4 heads in PSUM, evict bf16, DMA)

Phase order is P(b0) P(b1) A(b0) A(b1) with double-buffered qT/kT/vsb so the PE
never sees a projection<->attention boundary stall. Output-projection matmul
"jobs" are popped from a queue inside the attention streams to keep the in-order
PE queue dense while ACT works through the exps.

Scheduling notes (hard-won against the in-order engine queues):
 - PSUM banks are freed by fast ACT copies (~0.8us) rather than the rope math;
   the head-major projection tail staggers the pq stops so those copies
   pipeline against remaining matmuls and the next chunk never stalls.
 - The per-chunk v transpose (DMA crossbar) and the last chunk's ropes are
   emission-deferred so the scheduler cannot slot them ahead of bank-freeing
   copies / first attention mask-muls on their engine queues.
 - x, weights, and out use tiled/partition-major DRAM layouts so every DMA is
   contiguous per partition (2KB+ lines).
"""

import os
from collections import deque
from contextlib import ExitStack

import numpy as np
import ml_dtypes

import concourse.bacc as bacc
import concourse.mybir as mybir
import concourse.tile as tile

BF = mybir.dt.bfloat16
F32 = mybir.dt.float32
AFT = mybir.ActivationFunctionType

NCORES = 8
B, T, C = 2, 2048, 4096
H, KV, D = 32, 8, 128
QH = H // NCORES          # 4 q-heads per core
CT = C // 128             # 32 contraction tiles
NCH = T // 512            # 4 query chunks per batch
SCALE = 1.0 / float(np.sqrt(D))
EXP_BIAS = -10.0
ROPE_BASE = 10000.0

bf16 = ml_dtypes.bfloat16


def emit_program():
    nc = bacc.Bacc("TRN2", target_bir_lowering=False, debug=False,
                   num_devices=NCORES)

    # x tiled [c-tile, token-chunk, 128, 512] so every xt DMA is one
    # contiguous 128KB block; out tiled [row-tile, col-pair, 128, 1024] so
    # every store is one contiguous 256KB block (host reassembles)
    xT_d = nc.dram_tensor("xTt", [CT, B * T // 512, 128, 512], BF,
                          kind="ExternalInput").ap()
    wq_d = nc.dram_tensor("wq", [128, CT, QH * D], BF, kind="ExternalInput").ap()
    wk_d = nc.dram_tensor("wk", [128, CT, D], BF, kind="ExternalInput").ap()
    wv_d = nc.dram_tensor("wv", [128, CT, D], BF, kind="ExternalInput").ap()
    wo_d = nc.dram_tensor("woA", [128, QH, C], BF, kind="ExternalInput").ap()
    cos_d = nc.dram_tensor("cosT", [D, T], BF, kind="ExternalInput").ap()
    sin_d = nc.dram_tensor("sinTr", [D, T], BF, kind="ExternalInput").ap()
    alw_d = nc.dram_tensor("allowA", [128, 4, 512], BF, kind="ExternalInput").ap()
    out_d = nc.dram_tensor("out", [B * T // 128, C // 1024, 128, 1024], BF,
                           kind="ExternalOutput").ap()

    with tile.TileContext(nc) as tc, ExitStack() as ctx:
        const = ctx.enter_context(tc.tile_pool(name="const", bufs=1))
        act = ctx.enter_context(tc.tile_pool(name="act", bufs=1))
        work = ctx.enter_context(tc.tile_pool(name="work", bufs=1))

        # weights + tables on the gpsimd DMA queue so they never sit ahead
        # of the xt activation loads (sync queue); fine-grained groups so the
        # first projection matmuls only wait on ~0.4MB. alw/wo are gated to
        # load after P(b0) (they are only read in the attention phase).
        wq_sb = const.tile([128, CT, QH * D], BF)
        wk_sb = const.tile([128, CT, D], BF)
        wv_sb = const.tile([128, CT, D], BF)
        cos_sb = const.tile([D, T], BF)
        sin_sb = const.tile([D, T], BF)
        # finer groups early so the first matmuls start within ~2us, and the
        # rope tables only load once half the weights are in
        for g0, g1 in ((0, 1), (1, 2), (2, 4), (4, 8), (8, 16), (16, 24), (24, 32)):
            s = slice(g0, g1)
            nc.gpsimd.dma_start(wq_sb[:, s, :], wq_d[:, s, :])
            nc.gpsimd.dma_start(wk_sb[:, s, :], wk_d[:, s, :])
            nc.gpsimd.dma_start(wv_sb[:, s, :], wv_d[:, s, :])
            if g1 == 24:
                nc.gpsimd.dma_start(cos_sb[:], cos_d)
                nc.gpsimd.dma_start(sin_sb[:], sin_d)
        alw_sb = const.tile([128, 4, 512], BF)
        wo_sb = const.tile([128, QH, C], BF)
        onesbf_sb = const.tile([128, 128], BF)
        nc.gpsimd.memset(onesbf_sb[:], 1.0)
        bias_sb = const.tile([128, 1], F32)
        nc.gpsimd.memset(bias_sb[:], EXP_BIAS)

        def rope_sb(dst, src, cs):
            # dst = src * cos + swap_halves(src) * sin_rot   (all bf16 SBUF so
            # DVE runs in 2x/4x perf modes; src was evicted from PSUM by ACT)
            sw = work.tile([128, 512], BF, tag="sw", bufs=2, name="sw")
            nc.vector.tensor_copy(sw[0:64, :], src[64:128, :])
            nc.vector.tensor_copy(sw[64:128, :], src[0:64, :])
            nc.vector.tensor_mul(sw[:], sw[:], sin_sb[:, cs])
            cst = work.tile([128, 512], BF, tag="cst", bufs=2, name="cst")
            nc.vector.tensor_mul(cst[:], src[:], cos_sb[:, cs])
            nc.vector.tensor_add(dst, cst[:], sw[:])

        tr_pending = deque()
        rope_pending = deque()

        def flush_tr(n):
            for _ in range(min(n, len(tr_pending))):
                tr_pending.popleft()()

        def flush_rope(n):
            for _ in range(min(n, len(rope_pending))):
                rope_pending.popleft()()

        def proj_batch(pp, b):
            qT = act.tile([D, QH, T], BF, tag="qT", bufs=2, name="qT")
            kT = act.tile([D, T], BF, tag="kT", bufs=2, name="kT")
            vT = act.tile([D, T], BF, tag="vT", bufs=1, name="vT")
            vsb = act.tile([128, T // 128, D], BF, tag="v", bufs=2, name="vsb")
            for jc in range(NCH):
                pq = [pp.tile([128, 512], F32, tag=f"pq{h}", name=f"pq{h}")
                      for h in range(QH)]
                pk = pp.tile([128, 512], F32, tag="pk", bufs=2, name="pk")
                pv = pp.tile([128, 512], F32, tag="pv", bufs=2, name="pv")
                # q matmuls run SKEW c-tiles behind k/v so the previous
                # chunk's pq bank evictions are hidden behind ready work;
                # deeper skew in the very first chunk relaxes the deadline on
                # the tail weight groups while HBM is still loading them
                SKEW = 4
                xts = {}
                cc = (b * T + 512 * jc) // 512

                def q_mms(cq, h):
                    nc.tensor.matmul(
                        pq[h][:], wq_sb[:, cq, 128 * h:128 * (h + 1)],
                        xts[cq][:], start=cq == 0, stop=cq == CT - 1)

                for ci in range(CT):
                    # flush the previous chunk's deferred v-transpose here so
                    # it sits behind this chunk's first xt loads on the sync
                    # queue (emitted at the top it would head-of-line block
                    # the prefetch while waiting for the vT eviction)
                    if ci == 8:
                        flush_tr(4)
                    xt = work.tile([128, 512], BF, tag="xt", bufs=17, name="xt")
                    xts[ci] = xt
                    nc.sync.dma_start(xt[:], xT_d[ci, cc, :, :])
                    st, sp = ci == 0, ci == CT - 1
                    nc.tensor.matmul(pk[:], wk_sb[:, ci, :], xt[:],
                                     start=st, stop=sp)
                    nc.tensor.matmul(pv[:], wv_sb[:, ci, :], xt[:],
                                     start=st, stop=sp)
                    if ci >= SKEW:
                        for h in range(QH):
                            q_mms(ci - SKEW, h)
                        del xts[ci - SKEW]
                cs = slice(512 * jc, 512 * (jc + 1))
                # fast ACT copies free the PSUM banks so the next chunk's
                # matmuls never wait on the rope math (which runs SBUF-side
                # on DVE afterwards); kraw/vT copies overlap the q tail, and
                # the head-major tail staggers the pq stops so the qraw
                # copies pipeline against remaining tail matmuls
                kraw = work.tile([128, 512], BF, tag="kraw", bufs=2,
                                 name="kraw")
                nc.scalar.copy(kraw[:], pk[:])
                nc.scalar.copy(vT[:, cs], pv[:])
                qraws = []
                for h in range(QH):
                    for cq in range(CT - SKEW, CT):
                        q_mms(cq, h)
                    qraw = work.tile([128, 512], BF, tag="qraw", bufs=5,
                                     name="qraw")
                    nc.scalar.copy(qraw[:], pq[h][:])
                    qraws.append(qraw)
                # the very last chunk's ropes are deferred into the
                # attention phase: they are only read by A(b1), and emitted
                # here they would sit ahead of A(b0)'s mask-muls in the DVE
                # queue, stalling the first attention groups
                if b == 1 and jc == NCH - 1:
                    rope_pending.append(
                        lambda kT=kT, kraw=kraw, cs=cs: rope_sb(kT[:, cs], kraw, cs))
                    for h in range(QH):
                        rope_pending.append(
                            lambda qT=qT, h=h, q=qraws[h], cs=cs:
                                rope_sb(qT[:, h, cs], q, cs))
                else:
                    rope_sb(kT[:, cs], kraw, cs)
                    for h in range(QH):
                        rope_sb(qT[:, h, cs], qraws[h], cs)
                # v chunk -> [t, d] tiles via the DMA crossbar: one transpose
                # per chunk ([128,512] -> [128,4,128] maps vsb[p,k,d] =
                # vT[d,128k+p]), on the sync hwdge queue so its descriptor
                # cost never delays the ACT bank-freeing copies; deferred one
                # chunk so it cannot sit ahead of this chunk's copies
                tr_pending.append(
                    lambda jc=jc, cs=cs, vsb=vsb, vT=vT:
                        nc.sync.dma_start_transpose(
                            vsb[:, 4 * jc:4 * jc + 4, :], vT[:, cs]))
            return qT, kT, vsb

        with tc.tile_pool(name="pproj", bufs=1, space="PSUM") as pp:
            acts0 = proj_batch(pp, 0)
            # gate the wo/alw loads on a DVE op emitted here so the DMA
            # cannot start before P(b0) finishes -- keeps the HBM free for
            # the xt stream during the warmup chunks (both are only read in
            # the attention phase)
            nc.vector.memset(wo_sb[:, 0, 0:8], 0.0)
            nc.vector.memset(alw_sb[:, 0, 0:8], 0.0)
            nc.gpsimd.dma_start(alw_sb[:], alw_d)
            nc.gpsimd.dma_start(wo_sb[:], wo_d)
            acts1 = proj_batch(pp, 1)

        # ---- attention + output projection ----
        with tc.tile_pool(name="pattn", bufs=1, space="PSUM") as pa:
            wo_jobs = deque()
            drain_mode = [False]

            def make_wo_job(b, j, tl, op, yts):
                # one job covers two adjacent 512-col output slices so the
                # store DMA gets 2KB lines (one [128,1024] bf16 transfer)
                def job():
                    ob = work.tile([128, 1024], BF, tag="ob", bufs=3,
                                   name="ob")
                    for half in range(2):
                        o = 2 * op + half
                        ops = pa.tile([128, 512], F32, tag="ops", bufs=2,
                                      name="ops")
                        for h in range(QH):
                            nc.tensor.matmul(
                                ops[:], yts[h][:, 128 * tl:128 * (tl + 1)],
                                wo_sb[:, h, 512 * o:512 * (o + 1)],
                                start=h == 0, stop=h == QH - 1)
                        # spread the PSUM eviction across ACT and DVE:
                        # alternate except in the j==3 in-stream portion
                        # where ACT is saturated by exps (in the end-drain
                        # ACT is idle, so alternate there too)
                        if half == 0 and (j < 3 or drain_mode[0]):
                            nc.scalar.copy(ob[:, 0:512], ops[:])
                        else:
                            nc.vector.tensor_copy(ob[:, 512 * half:512 * (half + 1)], ops[:])
                    rt = (b * T + 512 * j + 128 * tl) // 128
                    nc.sync.dma_start(out_d[rt, op, :, :], ob[:])
                return job

            for b, (qT, kT, vsb) in ((0, acts0), (1, acts1)):
                for j in range(NCH):
                    yts = {}
                    for h in range(QH):
                        # sprinkle the last projection chunk's v transposes
                        # and deferred ropes into the early attention heads
                        flush_tr(1)
                        if not (b == 0 and j == 0 and h == 0):
                            flush_rope(1)
                        yps = pa.tile([128, 512], F32, tag="yps", bufs=1,
                                      name="yps")
                        K = 4 * j + 4
                        # pass 1: score matmuls stream; exp/mask/denominator
                        # trail on ACT/DVE. Diagonal tiles (o>=1) only touch
                        # their unmasked column range [128*o:512].
                        S = work.tile([128, 512], BF, tag="S", bufs=2,
                                      name="S")
                        pts = []
                        for k in range(K):
                            o = k - 4 * j
                            c0 = 128 * o if o > 0 else 0
                            sl = slice(c0, 512)
                            sps = pa.tile([128, 512], F32, tag="sps", bufs=4,
                                          name="sps")
                            nc.tensor.matmul(
                                sps[:, sl], kT[:, 128 * k:128 * (k + 1)],
                                qT[:, h, 512 * j + c0:512 * (j + 1)],
                                start=True, stop=True)
                            pt = work.tile([128, 512], BF, tag="pt", bufs=16,
                                           name="pt")
                            nc.scalar.activation(pt[:, sl], sps[:, sl], AFT.Exp,
                                                 bias=bias_sb[:], scale=SCALE)
                            if o >= 0:
                                nc.vector.tensor_mul(pt[:, sl], pt[:, sl],
                                                     alw_sb[:, o, sl])
                            if k == 0:
                                nc.vector.tensor_copy(S[:], pt[:])
                            else:
                                nc.vector.tensor_add(S[:, sl], S[:, sl],
                                                     pt[:, sl])
                            pts.append((pt, sl))
                            if wo_jobs:
                                wo_jobs.popleft()()
                        # pass 2: attn@v accumulation; k=0 always covers the
                        # full 512 columns so the start-matmul initializes the
                        # whole bank
                        for k, (pt, sl) in enumerate(pts):
                            nc.tensor.matmul(yps[:, sl], vsb[:, k, :],
                                             pt[:, sl],
                                             start=(k == 0), stop=(k == K - 1))
                            if wo_jobs:
                                wo_jobs.popleft()()
                        dns = pa.tile([128, 512], F32, tag="dns", bufs=1,
                                      name="dns")
                        nc.tensor.matmul(dns[:], onesbf_sb[:], S[:],
                                         start=True, stop=True)
                        rec = work.tile([128, 512], F32, tag="rec", bufs=1,
                                        name="rec")
                        nc.vector.reciprocal_approx_fast(rec[:], dns[:])
                        yt = work.tile([128, 512], BF, tag="yt", bufs=8,
                                       name="yt")
                        nc.vector.tensor_mul(yt[:], yps[:], rec[:])
                        yts[h] = yt
                    for tl in range(4):
                        for op in range(C // 1024):
                            wo_jobs.append(make_wo_job(b, j, tl, op, yts))
            drain_mode[0] = True
            while wo_jobs:
                wo_jobs.popleft()()

    nc.compile()
    return nc


def host_prep(inputs):
    x = np.asarray(inputs["x"], np.float32)
    mask = np.asarray(inputs["mask"], np.float32)
    wq = np.asarray(inputs["wq"], np.float32)
    wk = np.asarray(inputs["wk"], np.float32)
    wv = np.asarray(inputs["wv"], np.float32)
    wo = np.asarray(inputs["wo"], np.float32)

    xT = x.reshape(B * T, C).T  # [C, B*T]
    xTt = np.ascontiguousarray(
        xT.reshape(CT, 128, B * T // 512, 512).transpose(0, 2, 1, 3)
    ).astype(bf16)  # [ci, chunk, 128, 512] contiguous tiles
    inv = 1.0 / (ROPE_BASE ** (np.arange(0, D, 2, dtype=np.float64) / D))
    freqs = np.arange(T, dtype=np.float64)[:, None] * inv[None, :] * B
    emb = np.concatenate([freqs, freqs], axis=-1)       # [T, D]
    cosT = np.cos(emb).T.astype(np.float32).astype(bf16)
    sinT = np.sin(emb).T.astype(np.float32)
    sinT[: D // 2] *= -1.0
    sinTr = sinT.astype(bf16)
    # allow[p, o, jj] = 1 - mask[jj, 128*o + p]  (from the actual mask input)
    allowA = np.ascontiguousarray(
        np.stack([(1.0 - mask[0:512, 128 * o:128 * (o + 1)]).T
                  for o in range(4)], axis=1)).astype(bf16)   # [128, 4, 512]

    common = dict(xTt=xTt, cosT=cosT, sinTr=sinTr, allowA=allowA)
    in_maps = []
    for c in range(NCORES):
        m = dict(common)
        m["wq"] = np.ascontiguousarray(
            wq[:, 512 * c:512 * (c + 1)].reshape(CT, 128, QH * D)
            .transpose(1, 0, 2)).astype(bf16)
        m["wk"] = np.ascontiguousarray(
            wk[:, 128 * c:128 * (c + 1)].reshape(CT, 128, D)
            .transpose(1, 0, 2)).astype(bf16)
        m["wv"] = np.ascontiguousarray(
            wv[:, 128 * c:128 * (c + 1)].reshape(CT, 128, D)
            .transpose(1, 0, 2)).astype(bf16)
        m["woA"] = np.ascontiguousarray(
            wo[512 * c:512 * (c + 1), :].reshape(QH, 128, C)
            .transpose(1, 0, 2)).astype(bf16)
        in_maps.append(m)
    return in_maps


def kernel(**inputs) -> np.ndarray:
    from concourse.bass_utils import run_bass_kernel_spmd

    in_maps = host_prep(inputs)
    nc = emit_program()
    trace = bool(os.environ.get("BASS_KERNEL_TRACE"))
    res = run_bass_kernel_spmd(nc, in_maps, core_ids=list(range(NCORES)),
                               trace=trace)
    if trace and res.exec_time_ns is not None:
        print(f"HW exec time: {res.exec_time_ns} ns")
        if res.instructions_and_trace is not None:
            print("trace:", res.instructions_and_trace[1])
    total = np.zeros((B * T // 128, C // 1024, 128, 1024), np.float32)
    for r in res.results:
        total += np.asarray(r["out"], dtype=np.float32)
    # untile [row-tile, col-pair, 128, 1024] -> [B*T, C]
    full = total.transpose(0, 2, 1, 3).reshape(B * T, C)
    return np.ascontiguousarray(full).reshape(B, T, C)


# revision 34
# speedup vs baseline: 1.0603x; 1.0466x over previous
"""Trainium2 Bass kernel for GQA attention (B=2, T=2048, C=4096, H=32, KV=8, D=128)
with RoPE and causal mask.

Sharding: tensor-parallel over heads across 8 cores. Each core owns 4 Q heads and
their shared KV head: projects q/k/v for those heads, runs causal attention, and
computes a partial output projection; the host sums the 8 partials (bf16 partials,
f32 accumulation on host).

All on-chip layouts are transposed ([feature, token]) so every matmul consumes
natural slices:
  qT/kT/vT = W^T @ x  via lhsT=W-tile [128c, cols], rhs=xT-tile [128c, 512t]
  sT[tk, tq] = kT-tile^T @ qT-chunk   (per 128-row key tile x 512-col query chunk;
               diagonal tiles stream only their unmasked column range)
  pT = exp(sT/sqrt(D) - 10) on ACT; strictly-causal-upper tiles skipped entirely
  S  = sum_k pT  accumulated on DVE (bf16) -> one ones-matmul per (b,h,j) gives
       the softmax denominator broadcast in PSUM (replaces a ones-matmul per tile)
  yT[d, tq] += v-tile^T @ pT          (v laid out [t, d] via DMA-crossbar transpose)
  out[tq, :] += yT_h^T @ wo_h         (accumulate 4 heads in PSUM, evict bf16, DMA)

Phase order is P(b0) P(b1) A(b0) A(b1) with double-buffered qT/kT/vsb so the PE
never sees a projection<->attention boundary stall. Output-projection matmul
"jobs" are popped from a queue inside the attention streams to keep the in-order
PE queue dense while ACT works through the exps.

Scheduling notes (hard-won against the in-order engine queues):
 - PSUM banks are freed by fast ACT copies (~0.8us) rather than the rope math;
   the head-major projection tail staggers the pq stops so those copies
   pipeline against remaining matmuls and the next chunk never stalls.
 - The per-chunk v transpose (DMA crossbar) and the last chunk's ropes are
   emission-deferred so the scheduler cannot slot them ahead of bank-freeing
   copies / first attention mask-muls on their engine queues.
 - x, weights, and out use tiled/partition-major DRAM layouts so every DMA is
   contiguous per partition (2KB+ lines).
"""

import os
from collections import deque
from contextlib import ExitStack

import numpy as np
import ml_dtypes

import concourse.bacc as bacc
import concourse.mybir as mybir
import concourse.tile as tile

BF = mybir.dt.bfloat16
F32 = mybir.dt.float32
AFT = mybir.ActivationFunctionType

NCORES = 8
B, T, C = 2, 2048, 4096
H, KV, D = 32, 8, 128
QH = H // NCORES          # 4 q-heads per core
CT = C // 128             # 32 contraction tiles
NCH = T // 512            # 4 query chunks per batch
SCALE = 1.0 / float(np.sqrt(D))
EXP_BIAS = -10.0
ROPE_BASE = 10000.0

bf16 = ml_dtypes.bfloat16


def emit_program():
    nc = bacc.Bacc("TRN2", target_bir_lowering=False, debug=False,
                   num_devices=NCORES)

    # x tiled [c-tile, token-chunk, 128, 512] so every xt DMA is one
    # contiguous 128KB block; out tiled [row-tile, col-pair, 128, 1024] so
    # every store is one contiguous 256KB block (host reassembles)
    xT_d = nc.dram_tensor("xTt", [CT, B * T // 512, 128, 512], BF,
                          kind="ExternalInput").ap()
    wq_d = nc.dram_tensor("wq", [128, CT, QH * D], BF, kind="ExternalInput").ap()
    wk_d = nc.dram_tensor("wk", [128, CT, D], BF, kind="ExternalInput").ap()
    wv_d = nc.dram_tensor("wv", [128, CT, D], BF, kind="ExternalInput").ap()
    wo_d = nc.dram_tensor("woA", [128, QH, C], BF, kind="ExternalInput").ap()
    cos_d = nc.dram_tensor("cosT", [D, T], BF, kind="ExternalInput").ap()
    sin_d = nc.dram_tensor("sinTr", [D, T], BF, kind="ExternalInput").ap()
    alw_d = nc.dram_tensor("allowA", [128, 4, 512], BF, kind="ExternalInput").ap()
    out_d = nc.dram_tensor("out", [B * T // 128, C // 1024, 128, 1024], BF,
                           kind="ExternalOutput").ap()

    with tile.TileContext(nc) as tc, ExitStack() as ctx:
        const = ctx.enter_context(tc.tile_pool(name="const", bufs=1))
        act = ctx.enter_context(tc.tile_pool(name="act", bufs=1))
        work = ctx.enter_context(tc.tile_pool(name="work", bufs=1))

        # weights + tables on the gpsimd DMA queue so they never sit ahead
        # of the xt activation loads (sync queue); fine-grained groups so the
        # first projection matmuls only wait on ~0.4MB. alw/wo are gated to
        # load after P(b0) (they are only read in the attention phase).
        wq_sb = const.tile([128, CT, QH * D], BF)
        wk_sb = const.tile([128, CT, D], BF)
        wv_sb = const.tile([128, CT, D], BF)
        cos_sb = const.tile([D, T], BF)
        sin_sb = const.tile([D, T], BF)
        # finer groups early so the first matmuls start within ~2us, and the
        # rope tables only load once half the weights are in
        for g0, g1 in ((0, 1), (1, 2), (2, 4), (4, 8), (8, 16), (16, 24), (24, 32)):
            s = slice(g0, g1)
            nc.gpsimd.dma_start(wq_sb[:, s, :], wq_d[:, s, :])
            nc.gpsimd.dma_start(wk_sb[:, s, :], wk_d[:, s, :])
            nc.gpsimd.dma_start(wv_sb[:, s, :], wv_d[:, s, :])
            if g1 == 24:
                nc.gpsimd.dma_start(cos_sb[:], cos_d)
                nc.gpsimd.dma_start(sin_sb[:], sin_d)
        alw_sb = const.tile([128, 4, 512], BF)
        wo_sb = const.tile([128, QH, C], BF)
        onesbf_sb = const.tile([128, 128], BF)
        nc.gpsimd.memset(onesbf_sb[:], 1.0)
        bias_sb = const.tile([128, 1], F32)
        nc.gpsimd.memset(bias_sb[:], EXP_BIAS)

        def rope_sb(dst, src, cs):
            # dst = src * cos + swap_halves(src) * sin_rot   (all bf16 SBUF so
            # DVE runs in 2x/4x perf modes; src was evicted from PSUM by ACT)
            sw = work.tile([128, 512], BF, tag="sw", bufs=2, name="sw")
            nc.vector.tensor_copy(sw[0:64, :], src[64:128, :])
            nc.vector.tensor_copy(sw[64:128, :], src[0:64, :])
            nc.vector.tensor_mul(sw[:], sw[:], sin_sb[:, cs])
            cst = work.tile([128, 512], BF, tag="cst", bufs=2, name="cst")
            nc.vector.tensor_mul(cst[:], src[:], cos_sb[:, cs])
            nc.vector.tensor_add(dst, cst[:], sw[:])

        tr_pending = deque()
        rope_pending = deque()

        def flush_tr(n):
            for _ in range(min(n, len(tr_pending))):
                tr_pending.popleft()()

        def flush_rope(n):
            for _ in range(min(n, len(rope_pending))):
                rope_pending.popleft()()

        def proj_batch(pp, b):
            qT = act.tile([D, QH, T], BF, tag="qT", bufs=2, name="qT")
            kT = act.tile([D, T], BF, tag="kT", bufs=2, name="kT")
            vT = act.tile([D, T], BF, tag="vT", bufs=1, name="vT")
            vsb = act.tile([128, T // 128, D], BF, tag="v", bufs=2, name="vsb")
            for jc in range(NCH):
                pq = [pp.tile([128, 512], F32, tag=f"pq{h}", name=f"pq{h}")
                      for h in range(QH)]
                pk = pp.tile([128, 512], F32, tag="pk", bufs=2, name="pk")
                pv = pp.tile([128, 512], F32, tag="pv", bufs=2, name="pv")
                # q matmuls run SKEW c-tiles behind k/v so the previous
                # chunk's pq bank evictions are hidden behind ready work;
                # deeper skew in the very first chunk relaxes the deadline on
                # the tail weight groups while HBM is still loading them
                SKEW = 4
                xts = {}
                cc = (b * T + 512 * jc) // 512

                def q_mms(cq, h):
                    nc.tensor.matmul(
                        pq[h][:], wq_sb[:, cq, 128 * h:128 * (h + 1)],
                        xts[cq][:], start=cq == 0, stop=cq == CT - 1)

                for ci in range(CT):
                    # flush the previous chunk's deferred v-transpose here so
                    # it sits behind this chunk's first xt loads on the sync
                    # queue (emitted at the top it would head-of-line block
                    # the prefetch while waiting for the vT eviction)
                    if ci == 8:
                        flush_tr(4)
                    xt = work.tile([128, 512], BF, tag="xt", bufs=17, name="xt")
                    xts[ci] = xt
                    nc.sync.dma_start(xt[:], xT_d[ci, cc, :, :])
                    st, sp = ci == 0, ci == CT - 1
                    nc.tensor.matmul(pk[:], wk_sb[:, ci, :], xt[:],
                                     start=st, stop=sp)
                    nc.tensor.matmul(pv[:], wv_sb[:, ci, :], xt[:],
                                     start=st, stop=sp)
                    if ci >= SKEW:
                        for h in range(QH):
                            q_mms(ci - SKEW, h)
                        del xts[ci - SKEW]
                cs = slice(512 * jc, 512 * (jc + 1))
                # fast ACT copies free the PSUM banks so the next chunk's
                # matmuls never wait on the rope math (which runs SBUF-side
                # on DVE afterwards); kraw/vT copies overlap the q tail, and
                # the head-major tail staggers the pq stops so the qraw
                # copies pipeline against remaining tail matmuls
                kraw = work.tile([128, 512], BF, tag="kraw", bufs=2,
                                 name="kraw")
                nc.scalar.copy(kraw[:], pk[:])
                nc.scalar.copy(vT[:, cs], pv[:])
                qraws = []
                for h in range(QH):
                    for cq in range(CT - SKEW, CT):
                        q_mms(cq, h)
                    qraw = work.tile([128, 512], BF, tag="qraw", bufs=5,
                                     name="qraw")
                    nc.scalar.copy(qraw[:], pq[h][:])
                    qraws.append(qraw)
                # the very last chunk's ropes are deferred into the
                # attention phase: they are only read by A(b1), and emitted
                # here they would sit ahead of A(b0)'s mask-muls in the DVE
                # queue, stalling the first attention groups
                if b == 1 and jc == NCH - 1:
                    rope_pending.append(
                        lambda kT=kT, kraw=kraw, cs=cs: rope_sb(kT[:, cs], kraw, cs))
                    for h in range(QH):
                        rope_pending.append(
                            lambda qT=qT, h=h, q=qraws[h], cs=cs:
                                rope_sb(qT[:, h, cs], q, cs))
                else:
                    rope_sb(kT[:, cs], kraw, cs)
                    for h in range(QH):
                        rope_sb(qT[:, h, cs], qraws[h], cs)
                # v chunk -> [t, d] tiles via the DMA crossbar: one transpose
                # per chunk ([128,512] -> [128,4,128] maps vsb[p,k,d] =
                # vT[d,128k+p]), on the sync hwdge queue so its descriptor
                # cost never delays the ACT bank-freeing copies; deferred one
                # chunk so it cannot sit ahead of this chunk's copies
                tr_pending.append(
                    lambda jc=jc, cs=cs, vsb=vsb, vT=vT:
                        nc.sync.dma_start_transpose(
                            vsb[:, 4 * jc:4 * jc + 4, :], vT[:, cs]))
            return qT, kT, vsb

        with tc.tile_pool(name="pproj", bufs=1, space="PSUM") as pp:
            acts0 = proj_batch(pp, 0)
            # gate the wo/alw loads on a DVE op emitted here so the DMA
            # cannot start before P(b0) finishes -- keeps the HBM free for
            # the xt stream during the warmup chunks (both are only read in
            # the attention phase)
            nc.vector.memset(wo_sb[:, 0, 0:8], 0.0)
            nc.vector.memset(alw_sb[:, 0, 0:8], 0.0)
            nc.gpsimd.dma_start(alw_sb[:], alw_d)
            nc.gpsimd.dma_start(wo_sb[:], wo_d)
            acts1 = proj_batch(pp, 1)

        # ---- attention + output projection ----
        with tc.tile_pool(name="pattn", bufs=1, space="PSUM") as pa:
            wo_jobs = deque()
            drain_mode = [False]
            slot_ctr = [0]

            def maybe_pop(stride):
                # ration the output-projection jobs evenly across pass-1
                # slots so late heads keep PE filler between score matmuls
                # (a FIFO drain empties the queue in the first head and the
                # bare scores then outrun ACT's exps, stalling on sps WAR)
                slot_ctr[0] += 1
                if wo_jobs and slot_ctr[0] % stride == 0:
                    wo_jobs.popleft()()

            def make_wo_job(b, j, tl, op, yts):
                # one job covers two adjacent 512-col output slices so the
                # store DMA gets 2KB lines (one [128,1024] bf16 transfer)
                def job():
                    ob = work.tile([128, 1024], BF, tag="ob", bufs=3,
                                   name="ob")
                    for half in range(2):
                        o = 2 * op + half
                        ops = pa.tile([128, 512], F32, tag="ops", bufs=2,
                                      name="ops")
                        for h in range(QH):
                            nc.tensor.matmul(
                                ops[:], yts[h][:, 128 * tl:128 * (tl + 1)],
                                wo_sb[:, h, 512 * o:512 * (o + 1)],
                                start=h == 0, stop=h == QH - 1)
                        # spread the PSUM eviction across ACT and DVE
                        if half == 0:
                            nc.scalar.copy(ob[:, 0:512], ops[:])
                        else:
                            nc.vector.tensor_copy(ob[:, 512 * half:512 * (half + 1)], ops[:])
                    rt = (b * T + 512 * j + 128 * tl) // 128
                    nc.sync.dma_start(out_d[rt, op, :, :], ob[:])
                return job

            for b, (qT, kT, vsb) in ((0, acts0), (1, acts1)):
                for j in range(NCH):
                    yts = {}
                    for h in range(QH):
                        # sprinkle the last projection chunk's v transposes
                        # and deferred ropes into the early attention heads
                        flush_tr(1)
                        if not (b == 0 and j == 0 and h == 0):
                            flush_rope(1)
                        yps = pa.tile([128, 512], F32, tag="yps", bufs=1,
                                      name="yps")
                        K = 4 * j + 4
                        # pass 1: score matmuls stream; exp/mask/denominator
                        # trail on ACT/DVE. Diagonal tiles (o>=1) only touch
                        # their unmasked column range [128*o:512].
                        S = work.tile([128, 512], BF, tag="S", bufs=2,
                                      name="S")
                        pts = []
                        for k in range(K):
                            o = k - 4 * j
                            c0 = 128 * o if o > 0 else 0
                            sl = slice(c0, 512)
                            sps = pa.tile([128, 512], F32, tag="sps", bufs=4,
                                          name="sps")
                            nc.tensor.matmul(
                                sps[:, sl], kT[:, 128 * k:128 * (k + 1)],
                                qT[:, h, 512 * j + c0:512 * (j + 1)],
                                start=True, stop=True)
                            pt = work.tile([128, 512], BF, tag="pt", bufs=16,
                                           name="pt")
                            nc.scalar.activation(pt[:, sl], sps[:, sl], AFT.Exp,
                                                 bias=bias_sb[:], scale=SCALE)
                            if o >= 0:
                                nc.vector.tensor_mul(pt[:, sl], pt[:, sl],
                                                     alw_sb[:, o, sl])
                            if k == 0:
                                nc.vector.tensor_copy(S[:], pt[:])
                            else:
                                nc.vector.tensor_add(S[:, sl], S[:, sl],
                                                     pt[:, sl])
                            pts.append((pt, sl))
                            maybe_pop(max(1, K // 4))
                        # pass 2: attn@v accumulation; k=0 always covers the
                        # full 512 columns so the start-matmul initializes the
                        # whole bank
                        for k, (pt, sl) in enumerate(pts):
                            nc.tensor.matmul(yps[:, sl], vsb[:, k, :],
                                             pt[:, sl],
                                             start=(k == 0), stop=(k == K - 1))
                        dns = pa.tile([128, 512], F32, tag="dns", bufs=1,
                                      name="dns")
                        nc.tensor.matmul(dns[:], onesbf_sb[:], S[:],
                                         start=True, stop=True)
                        rec = work.tile([128, 512], F32, tag="rec", bufs=1,
                                        name="rec")
                        nc.vector.reciprocal_approx_fast(rec[:], dns[:])
                        yt = work.tile([128, 512], BF, tag="yt", bufs=8,
                                       name="yt")
                        nc.vector.tensor_mul(yt[:], yps[:], rec[:])
                        yts[h] = yt
                    for tl in range(4):
                        for op in range(C // 1024):
                            wo_jobs.append(make_wo_job(b, j, tl, op, yts))
            drain_mode[0] = True
            while wo_jobs:
                wo_jobs.popleft()()

    nc.compile()
    return nc


def host_prep(inputs):
    x = np.asarray(inputs["x"], np.float32)
    mask = np.asarray(inputs["mask"], np.float32)
    wq = np.asarray(inputs["wq"], np.float32)
    wk = np.asarray(inputs["wk"], np.float32)
    wv = np.asarray(inputs["wv"], np.float32)
    wo = np.asarray(inputs["wo"], np.float32)

    xT = x.reshape(B * T, C).T  # [C, B*T]
    xTt = np.ascontiguousarray(
        xT.reshape(CT, 128, B * T // 512, 512).transpose(0, 2, 1, 3)
    ).astype(bf16)  # [ci, chunk, 128, 512] contiguous tiles
    inv = 1.0 / (ROPE_BASE ** (np.arange(0, D, 2, dtype=np.float64) / D))
    freqs = np.arange(T, dtype=np.float64)[:, None] * inv[None, :] * B
    emb = np.concatenate([freqs, freqs], axis=-1)       # [T, D]
    cosT = np.cos(emb).T.astype(np.float32).astype(bf16)
    sinT = np.sin(emb).T.astype(np.float32)
    sinT[: D // 2] *= -1.0
    sinTr = sinT.astype(bf16)
    # allow[p, o, jj] = 1 - mask[jj, 128*o + p]  (from the actual mask input)
    allowA = np.ascontiguousarray(
        np.stack([(1.0 - mask[0:512, 128 * o:128 * (o + 1)]).T
                  for o in range(4)], axis=1)).astype(bf16)   # [128, 4, 512]

    common = dict(xTt=xTt, cosT=cosT, sinTr=sinTr, allowA=allowA)
    in_maps = []
    for c in range(NCORES):
        m = dict(common)
        m["wq"] = np.ascontiguousarray(
            wq[:, 512 * c:512 * (c + 1)].reshape(CT, 128, QH * D)
            .transpose(1, 0, 2)).astype(bf16)
        m["wk"] = np.ascontiguousarray(
            wk[:, 128 * c:128 * (c + 1)].reshape(CT, 128, D)
            .transpose(1, 0, 2)).astype(bf16)
        m["wv"] = np.ascontiguousarray(
            wv[:, 128 * c:128 * (c + 1)].reshape(CT, 128, D)
            .transpose(1, 0, 2)).astype(bf16)
        m["woA"] = np.ascontiguousarray(
            wo[512 * c:512 * (c + 1), :].reshape(QH, 128, C)
            .transpose(1, 0, 2)).astype(bf16)
        in_maps.append(m)
    return in_maps


def kernel(**inputs) -> np.ndarray:
    from concourse.bass_utils import run_bass_kernel_spmd

    in_maps = host_prep(inputs)
    nc = emit_program()
    trace = bool(os.environ.get("BASS_KERNEL_TRACE"))
    res = run_bass_kernel_spmd(nc, in_maps, core_ids=list(range(NCORES)),
                               trace=trace)
    if trace and res.exec_time_ns is not None:
        print(f"HW exec time: {res.exec_time_ns} ns")
        if res.instructions_and_trace is not None:
            print("trace:", res.instructions_and_trace[1])
    total = np.zeros((B * T // 128, C // 1024, 128, 1024), np.float32)
    for r in res.results:
        total += np.asarray(r["out"], dtype=np.float32)
    # untile [row-tile, col-pair, 128, 1024] -> [B*T, C]
    full = total.transpose(0, 2, 1, 3).reshape(B * T, C)
    return np.ascontiguousarray(full).reshape(B, T, C)
